# revision 22
# baseline (speedup 1.0000x reference)
"""GNN message passing (GraphConvolution) on 8 TRN2 NeuronCores.

reference:
    support = x @ W                                   # [N, H]
    msgs    = support[edge_src] * edge_w[:, None]     # [E, H]
    agg     = segment_sum(msgs, edge_dst, N)          # [N, H]
    out     = relu(agg + b)

Strategy (dst-node 1D sharding; sharded support build + AllGather):
  - Core c owns dst nodes [c*NPC, (c+1)*NPC).
  - Phase 1 is sharded: core c computes support rows for 1/8 of each
    32768-row src "run", then a per-run AllGather replicates the run's
    support block into every core's DRAM. Phase-2 gathers for run r wait
    only on AG_r, so later AGs overlap earlier phase-2 work.
  - Phase 2 (run-major): edges bucketed by (dst supertile of 512, src run).
    Each bucket is one gather call + one PSUM [128,512] accumulation:
      chunk order = [L0 (512-wide, start)] + pure chunks (128-wide, one
      dst subtile each) + [L1..Lk-1 (512-wide; last carries stop)].
    Pure chunks hold a fixed per-subtile edge capacity (maxed over cores
    stays implicit: capacity = floor(mean/128)*128, overflow spills to
    the shared leftover chunks). Trailing pad uses idx=-1 so the gather
    ucode trims it per-core (gpsimd descriptor-gen is the bottleneck).
  - Weighted one-hot built in ONE DVE op per chunk:
      tensor_scalar(out, iota, scalar1=dstloc[:,j], op0=is_equal,
                    scalar2=ew[:,j], op1=mult)
    iota/dstloc/ew staged fp16 (integers exact to 2048; bf16 is not
    exact past 256, which matters for 512-wide leftover indicators).
  - Gathers spread over 4 SWDGE queues (greedy balance) so all 4 Q7
    core pairs generate descriptors concurrently.
  - Per-supertile results accumulate across runs in SBUF (f32); epilogue
    relu(acc + b) on ScalarE -> outT [H, NPC] -> host transpose.
"""

import math
import os

import ml_dtypes
import numpy as np

import concourse.bass as bass
import concourse.mybir as mybir
import concourse.tile as tile
from concourse import bacc
from concourse.bass_utils import run_bass_kernel_spmd
from concourse.library_config import mlp as _mlp_lib

BF16 = ml_dtypes.bfloat16
F16 = np.float16
SUB = 128
SUPER = 512
CHUNK = 32768

N_NODES = 100000
NFEAT = 256
NHID = 128
N_CORES = 8
NPC = N_NODES // N_CORES  # 12500
NPAD = 102400  # 8 * 12800, and >= N_NODES, multiple of 512
SHARD = NPAD // N_CORES  # 12800 rows of support computed per core
N_RUNS = 4  # src runs of 32768 rows (last run: 4096 rows)
GBUFS = 6  # gather tile pool depth (also: calls 0..GBUFS-1 avoid -1 trim)
USE_AG = bool(int(os.environ.get("GNN_AG", "1")))  # sharded ph1 + AllGather
USE_TRIM = bool(int(os.environ.get("GNN_TRIM", "0")))  # -1 tail pad for per-core trim


def _ceil_div(a, b):
    return (a + b - 1) // b


def _run_rows(r):
    """Absolute support-row range of run r."""
    r0 = r * CHUNK
    r1 = min((r + 1) * CHUNK, NPAD)
    return r0, r1


def prepare(x, edge_src, edge_dst, edge_w, W, b):
    n_nodes, nfeat = x.shape
    nhid = W.shape[1]
    assert (n_nodes, nfeat, nhid) == (N_NODES, NFEAT, NHID)
    n_super = _ceil_div(NPC, SUPER)  # 25 (last width 212)

    src = np.asarray(edge_src).astype(np.int64)
    dst = np.asarray(edge_dst).astype(np.int64)
    ew = np.asarray(edge_w).astype(np.float32)

    core_of = dst // NPC
    per_core = []
    # counts[c, S, so, r]
    max_so = _ceil_div(SUPER, SUB)
    counts = np.zeros((N_CORES, n_super, max_so, N_RUNS), np.int64)
    for c in range(N_CORES):
        m = core_of == c
        s_c = src[m]
        d_c = dst[m] - c * NPC
        w_c = ew[m]
        S_c = d_c >> 9
        so_c = (d_c >> 7) - 4 * S_c
        r_c = s_c >> 15
        key = ((S_c * N_RUNS + r_c) * max_so) + so_c
        order = np.argsort(key, kind="stable")
        s_c, d_c, w_c = s_c[order], d_c[order], w_c[order]
        S_c, so_c, r_c = S_c[order], so_c[order], r_c[order]
        np.add.at(counts[c], (S_c, so_c, r_c), 1)
        per_core.append((s_c, d_c, w_c, S_c, so_c, r_c))

    # static schedule ---------------------------------------------------
    # n_pure[S, so, r] = floor(mean_c count / 128); capacity = 128*n_pure
    mean_cnt = counts.mean(axis=0)
    n_pure = np.floor(mean_cnt / 128).astype(np.int64)
    cap = n_pure * 128
    # leftover per core per (S, r) = sum_so max(0, cnt - cap)
    leftover = np.maximum(counts - cap[None], 0).sum(axis=2)  # [c, S, r]
    n_left = np.maximum(2, _ceil_div(leftover.max(axis=0), 128))  # [S, r]

    # program order: run-major
    calls = []  # dicts with static schedule
    chunk_off = 0
    for r in range(N_RUNS):
        for S in range(n_super):
            wS = min(SUPER, NPC - S * SUPER)
            n_sub_here = _ceil_div(wS, SUB)
            meta = []  # per chunk: ("L",) or ("P", so)
            meta.append(("L",))
            for so in range(n_sub_here):
                for _ in range(int(n_pure[S, so, r])):
                    meta.append(("P", so))
            for _ in range(int(n_left[S, r]) - 1):
                meta.append(("L",))
            calls.append(
                dict(
                    r=r,
                    S=S,
                    n_chunks=len(meta),
                    chunk_off=chunk_off,
                    meta=meta,
                    n_pure_tab=[int(n_pure[S, so, r]) for so in range(n_sub_here)],
                    n_sub=n_sub_here,
                )
            )
            chunk_off += len(meta)
    nchunk = chunk_off
    e_pad = nchunk * 128
    gmax = max(cl["n_chunks"] for cl in calls)

    # greedy queue balance by static index count
    qload = [0, 0, 0, 0]
    for cl in calls:
        q = min(range(4), key=lambda i: qload[i])
        cl["queue"] = q
        qload[q] += cl["n_chunks"]

    # per-core data fill ------------------------------------------------
    in_maps = []
    # xT (bf16, transposed, padded) built once; per-core column shards
    xT = np.zeros((NFEAT, NPAD), BF16)
    xT[:, :N_NODES] = np.asarray(x, np.float32).T.astype(BF16)
    w_bf = np.ascontiguousarray(np.asarray(W, np.float32).astype(BF16))
    bias = np.asarray(b, np.float32).reshape(nhid, 1).copy()
    iota = np.tile(np.arange(SUPER, dtype=F16)[None, :], (128, 1))

    # per-core xT shard: for r<3 rows [32768r+4096c, +4096); r=3 [98304+512c, +512)
    shard_cols = []
    for c in range(N_CORES):
        cols = []
        for r in range(N_RUNS):
            r0, r1 = _run_rows(r)
            sh = (r1 - r0) // N_CORES
            cols.append(np.arange(r0 + c * sh, r0 + (c + 1) * sh))
        shard_cols.append(np.concatenate(cols))

    for c in range(N_CORES):
        s_c, d_c, w_c, S_c, so_c, r_c = per_core[c]
        idx_pad = np.zeros(e_pad, np.int16)
        dl_pad = np.zeros(e_pad, np.float32)
        ew_pad = np.zeros(e_pad, np.float32)
        # bucket edge lists: edges already sorted by (S, r, so)
        # build index: for each (S, r): per so slices
        # compute group starts via counts
        starts = {}
        pos = 0
        # edges sorted by key (S * N_RUNS + r) * max_so + so
        order_key = (S_c * N_RUNS + r_c) * max_so + so_c
        # they are already in this sorted order
        uniq, first = np.unique(order_key, return_index=True)
        bounds = dict(zip(uniq.tolist(), first.tolist()))
        total = len(s_c)

        def seg(S, r, so):
            k = (S * N_RUNS + r) * max_so + so
            if k not in bounds:
                return np.empty(0, np.int64), np.empty(0, np.int64), np.empty(0, np.float32)
            a = bounds[k]
            # end = next existing key start
            ks = uniq.tolist()
            # bisect
            import bisect

            i = bisect.bisect_right(ks, k)
            bnd = first[i] if i < len(uniq) else total
            return s_c[a:bnd], d_c[a:bnd], w_c[a:bnd]

        call_i = 0
        for cl in calls:
            r, S = cl["r"], cl["S"]
            run0, _ = _run_rows(r)
            base = cl["chunk_off"] * 128
            n_chunks = cl["n_chunks"]
            # slot layout: chunk j occupies [base+128j, base+128j+128)
            # L0 = chunk 0; pures follow; L_rest at the end
            lo_edges_s = []
            lo_edges_d = []
            lo_edges_w = []
            pj = 1  # next pure chunk index
            for so in range(cl["n_sub"]):
                es, ed, ewt = seg(S, r, so)
                capn = cl["n_pure_tab"][so] * 128
                take = min(len(es), capn)
                if capn > 0:
                    p0 = base + 128 * pj
                    idx_pad[p0 : p0 + take] = (es[:take] - run0).astype(np.int16)
                    dl_pad[p0 : p0 + take] = (ed[:take] & (SUB - 1)).astype(np.float32)
                    ew_pad[p0 : p0 + take] = ewt[:take].astype(np.float32)
                    pj += cl["n_pure_tab"][so]
                if len(es) > take:
                    lo_edges_s.append(es[take:])
                    lo_edges_d.append(ed[take:])
                    lo_edges_w.append(ewt[take:])
            # leftovers
            if lo_edges_s:
                ls = np.concatenate(lo_edges_s)
                ld = np.concatenate(lo_edges_d)
                lw = np.concatenate(lo_edges_w)
            else:
                ls = np.empty(0, np.int64)
                ld = np.empty(0, np.int64)
                lw = np.empty(0, np.float32)
            n_lo = len(ls)
            # L0 slots: [base, base+128); L_rest: [base+128*pj, base+128*n_chunks)
            k0 = min(n_lo, 128)
            idx_pad[base : base + k0] = (ls[:k0] - run0).astype(np.int16)
            dl_pad[base : base + k0] = (ld[:k0] - S * SUPER).astype(np.float32)
            ew_pad[base : base + k0] = lw[:k0].astype(np.float32)
            rest = n_lo - k0
            rest_base = base + 128 * pj
            rest_cap = (n_chunks - pj) * 128
            assert rest <= rest_cap, (c, call_i, rest, rest_cap)
            if rest > 0:
                idx_pad[rest_base : rest_base + rest] = (ls[k0:] - run0).astype(np.int16)
                dl_pad[rest_base : rest_base + rest] = (ld[k0:] - S * SUPER).astype(np.float32)
                ew_pad[rest_base : rest_base + rest] = lw[k0:].astype(np.float32)
            # trailing pad -> -1 so the gather ucode trims per-core.
            # Keep idx=0 for the first GBUFS calls so every gather pool
            # buffer gets fully written once (later stale reads are then
            # valid bf16, never NaN).
            if USE_TRIM and call_i >= GBUFS and rest < rest_cap:
                idx_pad[rest_base + rest : base + 128 * n_chunks] = -1
            call_i += 1

        in_maps.append(
            dict(
                xT=np.ascontiguousarray(xT[:, shard_cols[c]]) if USE_AG else xT,
                wmat=w_bf,
                bias=bias,
                iota=iota,
                idx=np.ascontiguousarray(np.tile(idx_pad.reshape(-1, 16).T, (8, 1))),
                dstloc=np.ascontiguousarray(dl_pad.reshape(-1, 128).T),
                ew=np.ascontiguousarray(ew_pad.reshape(-1, 128).T),
            )
        )

    cfg = dict(
        nfeat=NFEAT,
        nhid=NHID,
        n_cores=N_CORES,
        npc=NPC,
        n_super=n_super,
        nchunk=nchunk,
        e_pad=e_pad,
        gmax=gmax,
        calls=calls,
    )
    return cfg, in_maps


def build_bass(cfg):
    F, H = cfg["nfeat"], cfg["nhid"]
    KC = F // 128
    n_super = cfg["n_super"]
    XBLK = 512
    n_blocks = SHARD // XBLK  # 25

    nc = bacc.Bacc(
        "TRN2",
        target_bir_lowering=False,
        debug=False,
        enable_asserts=True,
        num_devices=cfg["n_cores"],
        num_swdge_queues=4,
    )
    f32, bf16, f16, i16 = (
        mybir.dt.float32,
        mybir.dt.bfloat16,
        mybir.dt.float16,
        mybir.dt.int16,
    )
    xT = nc.dram_tensor(
        "xT", [F, SHARD if USE_AG else NPAD], bf16, kind="ExternalInput"
    )
    wmat = nc.dram_tensor("wmat", [F, H], bf16, kind="ExternalInput")
    bias = nc.dram_tensor("bias", [H, 1], f32, kind="ExternalInput")
    iota = nc.dram_tensor("iota", [128, SUPER], f16, kind="ExternalInput")
    idx = nc.dram_tensor("idx", [128, cfg["e_pad"] // 16], i16, kind="ExternalInput")
    dstloc = nc.dram_tensor("dstloc", [128, cfg["nchunk"]], f32, kind="ExternalInput")
    ew = nc.dram_tensor("ew", [128, cfg["nchunk"]], f32, kind="ExternalInput")
    outT = nc.dram_tensor("outT", [H, cfg["npc"]], f32, kind="ExternalOutput")

    AF = mybir.ActivationFunctionType
    ALU = mybir.AluOpType
    rg = [list(range(cfg["n_cores"]))]

    with tile.TileContext(nc) as tc:
        with (
            tc.tile_pool(name="dram", bufs=1, space="DRAM") as dpool,
            tc.tile_pool(name="const", bufs=1) as cpool,
            tc.tile_pool(name="xt", bufs=3) as xpool,
            tc.tile_pool(name="sup", bufs=3) as spool,
            tc.tile_pool(name="gath", bufs=GBUFS) as gpool,
            tc.tile_pool(name="ind", bufs=8) as ipool,
            tc.tile_pool(name="meta", bufs=GBUFS) as mpool,
            tc.tile_pool(name="acc", bufs=1) as apool,
            tc.tile_pool(name="outb", bufs=2) as opool,
            tc.tile_pool(name="ps", bufs=8, space="PSUM") as ppool,
        ):
            nc.gpsimd.load_library(_mlp_lib)
            w_sb = cpool.tile([128, KC, H], bf16)
            nc.sync.dma_start(
                out=w_sb[:], in_=wmat.ap().rearrange("(c k) h -> k c h", k=128)
            )
            bias_sb = cpool.tile([H, 1], f32)
            nc.sync.dma_start(out=bias_sb[:], in_=bias.ap())
            iota_sb = cpool.tile([128, SUPER], f16)
            nc.sync.dma_start(out=iota_sb[:], in_=iota.ap())

            # DRAM scratch: per-run AllGather in/out
            ag_in = []
            support = []
            for r in range(N_RUNS):
                r0, r1 = _run_rows(r)
                sh = (r1 - r0) // N_CORES
                if USE_AG:
                    ag_in.append(
                        dpool.tile([sh, H], bf16, name=f"agin{r}", tag=f"agin{r}")
                    )
                support.append(
                    dpool.tile([r1 - r0, H], bf16, name=f"supp{r}", tag=f"supp{r}")
                )

            # warm the gather pool so stale reads are valid bf16
            for _ in range(GBUFS):
                gw = gpool.tile([128, cfg["gmax"], H], bf16, tag="gt")
                nc.vector.memset(gw[:], 0.0)

            # ---- phase 1 (sharded): support shard = xT_sh @ W, AG per run ----
            blk = 0
            for r in range(N_RUNS):
                r0, r1 = _run_rows(r)
                sh = (r1 - r0) // N_CORES if USE_AG else (r1 - r0)
                for bloc in range(sh // XBLK):
                    xts = []
                    for kc in range(KC):
                        xt = xpool.tile([128, XBLK], bf16, tag=f"xt{kc}")
                        nc.sync.dma_start(
                            out=xt[:],
                            in_=xT.ap()[
                                kc * 128 : (kc + 1) * 128,
                                blk * XBLK : (blk + 1) * XBLK,
                            ],
                        )
                        xts.append(xt)
                    st = spool.tile([128, XBLK], bf16)
                    ps1 = ppool.tile([128, XBLK], f32, tag="agg")
                    n_col = XBLK // 128
                    for i in range(n_col):
                        col = i * 128
                        for kc in range(KC):
                            # start marks the whole 2KB zero region pending-zero,
                            # so exactly one start (first mm) / stop (last mm).
                            nc.tensor.matmul(
                                ps1[:, col : col + 128],
                                xts[kc][:, col : col + 128],
                                w_sb[:, kc, :],
                                start=(i == 0 and kc == 0),
                                stop=(i == n_col - 1 and kc == KC - 1),
                            )
                    nc.scalar.activation(out=st[:], in_=ps1[:], func=AF.Copy)
                    wr_dst = ag_in[r] if USE_AG else support[r]
                    nc.sync.dma_start(
                        out=wr_dst[bloc * XBLK : (bloc + 1) * XBLK, :].rearrange(
                            "(i p) h -> p i h", p=128
                        ),
                        in_=st[:].rearrange("p (i h) -> p i h", h=H),
                    )
                    blk += 1
                if USE_AG:
                    nc.gpsimd.collective_compute(
                        "AllGather",
                        ALU.bypass,
                        replica_groups=rg,
                        ins=[ag_in[r].opt()],
                        outs=[support[r].opt()],
                    )

            # ---- phase 2 (run-major): gather + one-hot matmul + SBUF acc ----
            acc = apool.tile([128, n_super, SUPER], f32)
            nc.vector.memset(acc[:], 0.0)
            maxcalls = int(os.environ.get("GNN_MAXCALLS", str(len(cfg["calls"]))))
            for cl in cfg["calls"][:maxcalls]:
                r, S = cl["r"], cl["S"]
                Gc = cl["n_chunks"]
                L = Gc * 128
                c0 = cl["chunk_off"] * 8  # idx plane col = chunk_off*128/16
                idxt = mpool.tile([128, L // 16], i16, tag="idx")
                nc.sync.dma_start(out=idxt[:], in_=idx.ap()[:, c0 : c0 + L // 16])
                dlt = mpool.tile([128, Gc], f32, tag="dl")
                nc.sync.dma_start(
                    out=dlt[:],
                    in_=dstloc.ap()[:, cl["chunk_off"] : cl["chunk_off"] + Gc],
                )
                ewt = mpool.tile([128, Gc], f32, tag="ew")
                nc.sync.dma_start(
                    out=ewt[:], in_=ew.ap()[:, cl["chunk_off"] : cl["chunk_off"] + Gc]
                )
                gt = gpool.tile([128, cfg["gmax"], H], bf16, tag="gt")
                nc.gpsimd.dma_gather(
                    gt[:, :Gc, :],
                    support[r][:],
                    idxt[:],
                    L,
                    L,
                    H,
                    single_packet=False,
                    queue_num=cl["queue"],
                )
                ps = ppool.tile([128, SUPER], f32, tag="agg")
                last_j = Gc - 1
                for j, m in enumerate(cl["meta"]):
                    if m[0] == "P":
                        so = m[1]
                        ind = ipool.tile([128, SUB], bf16, tag="indp")
                        nc.vector.tensor_scalar(
                            out=ind[:],
                            in0=iota_sb[:, :SUB],
                            scalar1=dlt[:, j : j + 1],
                            scalar2=ewt[:, j : j + 1],
                            op0=ALU.is_equal,
                            op1=ALU.mult,
                        )
                        nc.tensor.matmul(
                            ps[:, so * SUB : (so + 1) * SUB],
                            gt[:, j, :],
                            ind[:],
                            start=False,
                            stop=False,
                        )
                    else:
                        ind = ipool.tile([128, SUPER], bf16, tag="indl")
                        nc.vector.tensor_scalar(
                            out=ind[:],
                            in0=iota_sb[:],
                            scalar1=dlt[:, j : j + 1],
                            scalar2=ewt[:, j : j + 1],
                            op0=ALU.is_equal,
                            op1=ALU.mult,
                        )
                        nc.tensor.matmul(
                            ps[:],
                            gt[:, j, :],
                            ind[:],
                            start=(j == 0),
                            stop=(j == last_j),
                        )
                if r == 0:
                    nc.vector.tensor_copy(acc[:, S, :], ps[:])
                else:
                    nc.vector.tensor_tensor(
                        out=acc[:, S, :], in0=acc[:, S, :], in1=ps[:], op=ALU.add
                    )

            # ---- epilogue: relu(acc + b) -> outT ----
            for S in range(n_super):
                wS = min(SUPER, cfg["npc"] - S * SUPER)
                ob = opool.tile([H, SUPER], f32)
                nc.scalar.activation(
                    out=ob[:, :wS],
                    in_=acc[:, S, :wS],
                    func=AF.Relu,
                    bias=bias_sb[:],
                    scale=1.0,
                )
                nc.sync.dma_start(
                    out=outT.ap()[:, S * SUPER : S * SUPER + wS], in_=ob[:, :wS]
                )
    nc.compile()
    return nc


def kernel(x, edge_src, edge_dst, edge_w, W, b):
    x = np.asarray(x)
    cfg, in_maps = prepare(x, edge_src, edge_dst, edge_w, W, b)
    nc = build_bass(cfg)
    want_trace = bool(int(os.environ.get("GNN_TRACE", "0")))
    core_ids = list(range(cfg["n_cores"]))
    if want_trace:
        try:
            res = run_bass_kernel_spmd(nc, in_maps, core_ids=core_ids, trace=True)
        except Exception as e:
            print(f"traced run failed ({e}); retrying without trace")
            res = run_bass_kernel_spmd(nc, in_maps, core_ids=core_ids, trace=False)
    else:
        res = run_bass_kernel_spmd(nc, in_maps, core_ids=core_ids, trace=False)
    kernel.last_result = res
    out = np.concatenate([r["outT"].T for r in res.results], axis=0)
    return np.ascontiguousarray(out).astype(np.float32)


kernel.last_result = None


# revision 24
# speedup vs baseline: 1.0886x; 1.0886x over previous
"""GNN message passing (GraphConvolution) on 8 TRN2 NeuronCores.

reference:
    support = x @ W                                   # [N, H]
    msgs    = support[edge_src] * edge_w[:, None]     # [E, H]
    agg     = segment_sum(msgs, edge_dst, N)          # [N, H]
    out     = relu(agg + b)

Strategy (dst-node 1D sharding; sharded support build + AllGather):
  - Core c owns dst nodes [c*NPC, (c+1)*NPC).
  - Phase 1 sharded: core c computes support rows [c*12800, (c+1)*12800)
    (bf16), one 8-way AllGather replicates the full table to every core.
  - Phase 2: edges bucketed by (dst subtile of 128, src run of 32768);
    per (supertile=512, run) call: gpsimd dma_gather of the bucket's
    support rows (128-row chunks, idx int16 per run), weighted one-hot
    indicator built on DVE (is_equal vs iota, then *ew), one TensorE
    matmul per 128-edge chunk accumulating psum[h, dst-subtile].
  - PSUM: one [128,512] bank per supertile, held across all 4 runs for
    a group of 8 supertiles at a time (start on the supertile's first
    chunk, stop on its last; the start marks the whole 2KB zero region).
  - Gathers spread over 4 SWDGE queues (greedy balance) so all 4 Q7
    core pairs generate descriptors concurrently (the bottleneck).
  - Optionally (GNN_EWSC percent of calls) the *ew multiply moves to
    ScalarE as a per-chunk Copy-activation with per-partition scale on
    the gathered tile, relieving DVE.
  - Epilogue relu(psum + b) on ScalarE -> outT [H, NPC] -> host transpose.
"""

import math
import os

import ml_dtypes
import numpy as np

import concourse.bass as bass
import concourse.mybir as mybir
import concourse.tile as tile
from concourse import bacc
from concourse.bass_utils import run_bass_kernel_spmd
from concourse.library_config import mlp as _mlp_lib

BF16 = ml_dtypes.bfloat16
SUB = 128
SUPER = 512
CHUNK = 32768

N_NODES = 100000
NFEAT = 256
NHID = 128
N_CORES = 8
NPC = N_NODES // N_CORES  # 12500
NPAD = 102400  # 8 * 12800, multiple of 512
SHARD = NPAD // N_CORES  # 12800 support rows computed per core
N_RUNS = 4
SGROUP = 8  # supertiles per psum-resident group
USE_AG = bool(int(os.environ.get("GNN_AG", "1")))
EWSC = int(os.environ.get("GNN_EWSC", "50"))  # % of calls with ew-mult on ScalarE


def _ceil_div(a, b):
    return (a + b - 1) // b


def _run_rows(r):
    r0 = r * CHUNK
    r1 = min((r + 1) * CHUNK, NPAD)
    return r0, r1


def prepare(x, edge_src, edge_dst, edge_w, W, b):
    n_nodes, nfeat = x.shape
    nhid = W.shape[1]
    assert (n_nodes, nfeat, nhid) == (N_NODES, NFEAT, NHID)
    n_super = _ceil_div(NPC, SUPER)  # 25
    max_so = SUPER // SUB  # 4

    src = np.asarray(edge_src).astype(np.int64)
    dst = np.asarray(edge_dst).astype(np.int64)
    ew = np.asarray(edge_w).astype(np.float32)

    core_of = dst // NPC
    per_core = []
    counts = np.zeros((N_CORES, n_super, max_so, N_RUNS), np.int64)
    for c in range(N_CORES):
        m = core_of == c
        s_c = src[m]
        d_c = dst[m] - c * NPC
        w_c = ew[m]
        S_c = d_c >> 9
        so_c = (d_c >> 7) - 4 * S_c
        r_c = s_c >> 15
        key = ((S_c * N_RUNS + r_c) * max_so) + so_c
        order = np.argsort(key, kind="stable")
        s_c, d_c, w_c, key = s_c[order], d_c[order], w_c[order], key[order]
        S_o, so_o, r_o = S_c[order], so_c[order], r_c[order]
        np.add.at(counts[c], (S_o, so_o, r_o), 1)
        per_core.append((s_c, d_c, w_c, key))

    # g_tab[S, so, r] = chunks for that bucket (maxed over cores)
    g_tab = _ceil_div(counts.max(axis=0), 128)
    for S in range(n_super):
        n_sub_here = _ceil_div(min(SUPER, NPC - S * SUPER), SUB)
        g_tab[S, 0, 0] = max(g_tab[S, 0, 0], 1)  # start anchor
        g_tab[S, n_sub_here - 1, 3] = max(g_tab[S, n_sub_here - 1, 3], 1)  # stop

    # program order: groups of SGROUP supertiles, runs inside, supertiles inside
    calls = []
    chunk_off = 0
    groups = [
        list(range(g, min(g + SGROUP, n_super))) for g in range(0, n_super, SGROUP)
    ]
    for grp in groups:
        for r in range(N_RUNS):
            for S in grp:
                wS = min(SUPER, NPC - S * SUPER)
                n_sub_here = _ceil_div(wS, SUB)
                meta = []
                for so in range(n_sub_here):
                    meta.extend([so] * int(g_tab[S, so, r]))
                if not meta:
                    continue
                calls.append(
                    dict(
                        r=r,
                        S=S,
                        n_chunks=len(meta),
                        chunk_off=chunk_off,
                        meta=meta,
                        first=(r == 0),
                        last=(r == 3),
                    )
                )
                chunk_off += len(meta)
    nchunk = chunk_off
    e_pad = nchunk * 128
    gmax = max(cl["n_chunks"] for cl in calls)

    # start/stop flags: start on S's first chunk (r==0 first call for S),
    # stop on S's last chunk (r==3 last call for S). Calls for S are in
    # r order, so 'first'/'last' flags above identify them.
    # greedy queue balance + ew-engine split
    qload = [0, 0, 0, 0]
    n_sc = 0
    for i, cl in enumerate(calls):
        q = min(range(4), key=lambda k: qload[k])
        cl["queue"] = q
        qload[q] += cl["n_chunks"]
        cl["ew_sc"] = (i * EWSC) // 100 != ((i + 1) * EWSC) // 100
        n_sc += int(cl["ew_sc"])

    in_maps = []
    xT = np.zeros((NFEAT, NPAD), BF16)
    xT[:, :N_NODES] = np.asarray(x, np.float32).T.astype(BF16)
    w_bf = np.ascontiguousarray(np.asarray(W, np.float32).astype(BF16))
    bias = np.asarray(b, np.float32).reshape(nhid, 1).copy()
    iota = np.tile(np.arange(SUB, dtype=np.float32).astype(BF16)[None, :], (128, 1))

    for c in range(N_CORES):
        s_c, d_c, w_c, key = per_core[c]
        idx_pad = np.zeros(e_pad, np.int16)
        dl_pad = np.zeros(e_pad, BF16)
        ew_pad = np.zeros(e_pad, np.float32)
        uniq, first = np.unique(key, return_index=True)
        uniq = uniq.tolist()
        first = first.tolist()
        total = len(s_c)

        import bisect

        def seg(S, r, so):
            k = (S * N_RUNS + r) * max_so + so
            i = bisect.bisect_left(uniq, k)
            if i >= len(uniq) or uniq[i] != k:
                return 0, 0
            a = first[i]
            bnd = first[i + 1] if i + 1 < len(uniq) else total
            return a, bnd

        for cl in calls:
            r, S = cl["r"], cl["S"]
            run0, _ = _run_rows(r)
            pos = cl["chunk_off"] * 128
            prev_so = -1
            for so in sorted(set(cl["meta"])):
                a, bnd = seg(S, r, so)
                n = bnd - a
                capn = cl["meta"].count(so) * 128
                assert n <= capn, (c, S, r, so, n, capn)
                p0 = cl["chunk_off"] * 128 + cl["meta"].index(so) * 128
                idx_pad[p0 : p0 + n] = (s_c[a:bnd] - run0).astype(np.int16)
                dl_pad[p0 : p0 + n] = (d_c[a:bnd] & (SUB - 1)).astype(BF16)
                ew_pad[p0 : p0 + n] = w_c[a:bnd].astype(np.float32)

        in_maps.append(
            dict(
                xT=np.ascontiguousarray(xT[:, c * SHARD : (c + 1) * SHARD])
                if USE_AG
                else xT,
                wmat=w_bf,
                bias=bias,
                iota=iota,
                idx=np.ascontiguousarray(np.tile(idx_pad.reshape(-1, 16).T, (8, 1))),
                dstloc=np.ascontiguousarray(dl_pad.reshape(-1, 128).T),
                ew=np.ascontiguousarray(ew_pad.reshape(-1, 128).T),
            )
        )

    cfg = dict(
        nfeat=NFEAT,
        nhid=NHID,
        n_cores=N_CORES,
        npc=NPC,
        n_super=n_super,
        nchunk=nchunk,
        e_pad=e_pad,
        gmax=gmax,
        calls=calls,
        groups=groups,
    )
    return cfg, in_maps


def build_bass(cfg):
    F, H = cfg["nfeat"], cfg["nhid"]
    KC = F // 128
    n_super = cfg["n_super"]
    XBLK = 512

    nc = bacc.Bacc(
        "TRN2",
        target_bir_lowering=False,
        debug=False,
        enable_asserts=True,
        num_devices=cfg["n_cores"],
        num_swdge_queues=4,
    )
    f32, bf16, i16 = mybir.dt.float32, mybir.dt.bfloat16, mybir.dt.int16
    xT = nc.dram_tensor(
        "xT", [F, SHARD if USE_AG else NPAD], bf16, kind="ExternalInput"
    )
    wmat = nc.dram_tensor("wmat", [F, H], bf16, kind="ExternalInput")
    bias = nc.dram_tensor("bias", [H, 1], f32, kind="ExternalInput")
    iota = nc.dram_tensor("iota", [128, SUB], bf16, kind="ExternalInput")
    idx = nc.dram_tensor("idx", [128, cfg["e_pad"] // 16], i16, kind="ExternalInput")
    dstloc = nc.dram_tensor("dstloc", [128, cfg["nchunk"]], bf16, kind="ExternalInput")
    ew = nc.dram_tensor("ew", [128, cfg["nchunk"]], f32, kind="ExternalInput")
    outT = nc.dram_tensor("outT", [H, cfg["npc"]], f32, kind="ExternalOutput")

    AF = mybir.ActivationFunctionType
    ALU = mybir.AluOpType
    rg = [list(range(cfg["n_cores"]))]

    with tile.TileContext(nc) as tc:
        with (
            tc.tile_pool(name="dram", bufs=1, space="DRAM") as dpool,
            tc.tile_pool(name="const", bufs=1) as cpool,
            tc.tile_pool(name="xt", bufs=3) as xpool,
            tc.tile_pool(name="sup", bufs=3) as spool,
            tc.tile_pool(name="gath", bufs=6) as gpool,
            tc.tile_pool(name="gtw", bufs=4) as wpool,
            tc.tile_pool(name="ind", bufs=4) as ipool,
            tc.tile_pool(name="meta", bufs=6) as mpool,
            tc.tile_pool(name="outb", bufs=2) as opool,
            tc.tile_pool(name="ps", bufs=8, space="PSUM") as ppool,
        ):
            nc.gpsimd.load_library(_mlp_lib)
            w_sb = cpool.tile([128, KC, H], bf16)
            nc.sync.dma_start(
                out=w_sb[:], in_=wmat.ap().rearrange("(c k) h -> k c h", k=128)
            )
            bias_sb = cpool.tile([H, 1], f32)
            nc.sync.dma_start(out=bias_sb[:], in_=bias.ap())
            iota_sb = cpool.tile([128, SUB], bf16)
            nc.sync.dma_start(out=iota_sb[:], in_=iota.ap())

            if USE_AG:
                ag_in = dpool.tile([SHARD, H], bf16, name="agin", tag="agin")
            support = dpool.tile([NPAD, H], bf16, name="supp", tag="supp")

            # ---- phase 1: support = x @ W (sharded when USE_AG) ----
            n_blocks = (SHARD if USE_AG else NPAD) // XBLK
            for blk in range(n_blocks):
                xts = []
                for kc in range(KC):
                    xt = xpool.tile([128, XBLK], bf16, tag=f"xt{kc}")
                    nc.sync.dma_start(
                        out=xt[:],
                        in_=xT.ap()[
                            kc * 128 : (kc + 1) * 128, blk * XBLK : (blk + 1) * XBLK
                        ],
                    )
                    xts.append(xt)
                st = spool.tile([128, XBLK], bf16)
                ps1 = ppool.tile([128, XBLK], f32, tag="agg")
                n_col = XBLK // 128
                for i in range(n_col):
                    col = i * 128
                    for kc in range(KC):
                        nc.tensor.matmul(
                            ps1[:, col : col + 128],
                            xts[kc][:, col : col + 128],
                            w_sb[:, kc, :],
                            start=(i == 0 and kc == 0),
                            stop=(i == n_col - 1 and kc == KC - 1),
                        )
                nc.scalar.activation(out=st[:], in_=ps1[:], func=AF.Copy)
                wr_dst = ag_in if USE_AG else support
                nc.sync.dma_start(
                    out=wr_dst[blk * XBLK : (blk + 1) * XBLK, :].rearrange(
                        "(i p) h -> p i h", p=128
                    ),
                    in_=st[:].rearrange("p (i h) -> p i h", h=H),
                )
            if USE_AG:
                nc.gpsimd.collective_compute(
                    "AllGather",
                    ALU.bypass,
                    replica_groups=rg,
                    ins=[ag_in.opt()],
                    outs=[support.opt()],
                )

            # ---- phase 2: per (supertile, run) gather + one-hot matmuls ----
            pss = {}
            call_idx = {}
            for cl in cfg["calls"]:
                call_idx.setdefault(cl["S"], []).append(cl)
            for cl in cfg["calls"]:
                r, S = cl["r"], cl["S"]
                run0, run1 = _run_rows(r)
                Gc = cl["n_chunks"]
                L = Gc * 128
                c0 = cl["chunk_off"] * 8
                idxt = mpool.tile([128, L // 16], i16, tag="idx")
                nc.sync.dma_start(out=idxt[:], in_=idx.ap()[:, c0 : c0 + L // 16])
                dlt = mpool.tile([128, Gc], bf16, tag="dl")
                nc.sync.dma_start(
                    out=dlt[:],
                    in_=dstloc.ap()[:, cl["chunk_off"] : cl["chunk_off"] + Gc],
                )
                ewt = mpool.tile([128, Gc], f32, tag="ew")
                nc.sync.dma_start(
                    out=ewt[:], in_=ew.ap()[:, cl["chunk_off"] : cl["chunk_off"] + Gc]
                )
                gt = gpool.tile([128, cfg["gmax"], H], bf16, tag="gt")
                nc.gpsimd.dma_gather(
                    gt[:, :Gc, :],
                    support[run0:run1, :],
                    idxt[:],
                    L,
                    L,
                    H,
                    single_packet=False,
                    queue_num=cl["queue"],
                )
                if cl["first"]:
                    pss[S] = ppool.tile([128, SUPER], f32, tag="agg", name=f"psS{S}")
                ps = pss[S]
                ind = ipool.tile([128, Gc, SUB], bf16, tag="ind")
                if cl["ew_sc"]:
                    # DVE: plain one-hot; ScalarE: scale gathered rows by ew
                    nc.vector.tensor_tensor(
                        out=ind[:],
                        in0=iota_sb[:][:, None, :].to_broadcast([128, Gc, SUB]),
                        in1=dlt[:][:, :, None].to_broadcast([128, Gc, SUB]),
                        op=ALU.is_equal,
                    )
                    gtw = wpool.tile([128, cfg["gmax"], H], bf16, tag="gtw")
                    for j in range(Gc):
                        nc.scalar.activation(
                            out=gtw[:, j, :],
                            in_=gt[:, j, :],
                            func=AF.Copy,
                            scale=ewt[:, j : j + 1],
                        )
                    mm_in = gtw
                else:
                    nc.vector.tensor_tensor(
                        out=ind[:],
                        in0=iota_sb[:][:, None, :].to_broadcast([128, Gc, SUB]),
                        in1=dlt[:][:, :, None].to_broadcast([128, Gc, SUB]),
                        op=ALU.is_equal,
                    )
                    nc.vector.tensor_tensor(
                        out=ind[:],
                        in0=ind[:],
                        in1=ewt[:][:, :, None].to_broadcast([128, Gc, SUB]),
                        op=ALU.mult,
                    )
                    mm_in = gt
                first_of_S = cl["first"]
                last_of_S = cl["last"]
                for j, so in enumerate(cl["meta"]):
                    nc.tensor.matmul(
                        ps[:, so * SUB : (so + 1) * SUB],
                        mm_in[:, j, :],
                        ind[:, j, :],
                        start=(first_of_S and j == 0),
                        stop=(last_of_S and j == Gc - 1),
                    )
                if last_of_S:
                    wS = min(SUPER, cfg["npc"] - S * SUPER)
                    ob = opool.tile([H, SUPER], f32)
                    nc.scalar.activation(
                        out=ob[:, :wS],
                        in_=ps[:, :wS],
                        func=AF.Relu,
                        bias=bias_sb[:],
                        scale=1.0,
                    )
                    nc.sync.dma_start(
                        out=outT.ap()[:, S * SUPER : S * SUPER + wS], in_=ob[:, :wS]
                    )
                    del pss[S]
    nc.compile()
    return nc


def kernel(x, edge_src, edge_dst, edge_w, W, b):
    x = np.asarray(x)
    cfg, in_maps = prepare(x, edge_src, edge_dst, edge_w, W, b)
    nc = build_bass(cfg)
    want_trace = bool(int(os.environ.get("GNN_TRACE", "0")))
    core_ids = list(range(cfg["n_cores"]))
    if want_trace:
        try:
            res = run_bass_kernel_spmd(nc, in_maps, core_ids=core_ids, trace=True)
        except Exception as e:
            print(f"traced run failed ({e}); retrying without trace")
            res = run_bass_kernel_spmd(nc, in_maps, core_ids=core_ids, trace=False)
    else:
        res = run_bass_kernel_spmd(nc, in_maps, core_ids=core_ids, trace=False)
    kernel.last_result = res
    out = np.concatenate([r["outT"].T for r in res.results], axis=0)
    return np.ascontiguousarray(out).astype(np.float32)


kernel.last_result = None


# revision 29
# speedup vs baseline: 1.1968x; 1.0994x over previous
"""GNN message passing (GraphConvolution) on 8 TRN2 NeuronCores.

reference:
    support = x @ W                                   # [N, H]
    msgs    = support[edge_src] * edge_w[:, None]     # [E, H]
    agg     = segment_sum(msgs, edge_dst, N)          # [N, H]
    out     = relu(agg + b)

Strategy (dst-node 1D sharding; sharded support build + AllGather):
  - Core c owns dst nodes [c*NPC, (c+1)*NPC).
  - Phase 1 sharded: core c computes support rows [c*12800, (c+1)*12800)
    (bf16), one 8-way AllGather replicates the full table to every core.
  - Phase 2: edges bucketed by (dst subtile of 128, src run of 32768);
    per (supertile=512, run) call: gpsimd dma_gather of the bucket's
    support rows (128-row chunks, idx int16 per run), weighted one-hot
    indicator built on DVE (is_equal vs iota, then *ew), one TensorE
    matmul per 128-edge chunk accumulating psum[h, dst-subtile].
  - PSUM: one [128,512] bank per supertile, held across all 4 runs for
    a group of 8 supertiles at a time (start on the supertile's first
    chunk, stop on its last; the start marks the whole 2KB zero region).
  - Gathers spread over 4 SWDGE queues (greedy balance) so all 4 Q7
    core pairs generate descriptors concurrently (the bottleneck).
  - Optionally (GNN_EWSC percent of calls) the *ew multiply moves to
    ScalarE as a per-chunk Copy-activation with per-partition scale on
    the gathered tile, relieving DVE.
  - Epilogue relu(psum + b) on ScalarE -> outT [H, NPC] -> host transpose.
"""

import math
import os

import ml_dtypes
import numpy as np

import concourse.bass as bass
import concourse.mybir as mybir
import concourse.tile as tile
from concourse import bacc
from concourse.bass_utils import run_bass_kernel_spmd
from concourse.library_config import mlp as _mlp_lib

BF16 = ml_dtypes.bfloat16
SUB = 128
SUPER = 512
CHUNK = 32768

N_NODES = 100000
NFEAT = 256
NHID = 128
N_CORES = 8
NPC = N_NODES // N_CORES  # 12500
NPAD = 102400  # 8 * 12800, multiple of 512
SHARD = NPAD // N_CORES  # 12800 support rows computed per core
N_RUNS = 4
SGROUP = 8  # supertiles per psum-resident group
USE_AG = bool(int(os.environ.get("GNN_AG", "1")))
EWSC = int(os.environ.get("GNN_EWSC", "50"))  # % of calls with ew-mult on ScalarE


def _ceil_div(a, b):
    return (a + b - 1) // b


def _run_rows(r):
    r0 = r * CHUNK
    r1 = min((r + 1) * CHUNK, NPAD)
    return r0, r1


def prepare(x, edge_src, edge_dst, edge_w, W, b):
    n_nodes, nfeat = x.shape
    nhid = W.shape[1]
    assert (n_nodes, nfeat, nhid) == (N_NODES, NFEAT, NHID)
    n_super = _ceil_div(NPC, SUPER)  # 25
    max_so = SUPER // SUB  # 4

    src = np.asarray(edge_src).astype(np.int64)
    dst = np.asarray(edge_dst).astype(np.int64)
    ew = np.asarray(edge_w).astype(np.float32)

    core_of = dst // NPC
    per_core = []
    counts = np.zeros((N_CORES, n_super, max_so, N_RUNS), np.int64)
    for c in range(N_CORES):
        m = core_of == c
        s_c = src[m]
        d_c = dst[m] - c * NPC
        w_c = ew[m]
        S_c = d_c >> 9
        so_c = (d_c >> 7) - 4 * S_c
        r_c = s_c >> 15
        key = ((S_c * N_RUNS + r_c) * max_so) + so_c
        order = np.argsort(key, kind="stable")
        s_c, d_c, w_c, key = s_c[order], d_c[order], w_c[order], key[order]
        S_o, so_o, r_o = S_c[order], so_c[order], r_c[order]
        np.add.at(counts[c], (S_o, so_o, r_o), 1)
        per_core.append((s_c, d_c, w_c, key))

    # g_tab[S, so, r] = chunks for that bucket (maxed over cores)
    g_tab = _ceil_div(counts.max(axis=0), 128)
    for S in range(n_super):
        n_sub_here = _ceil_div(min(SUPER, NPC - S * SUPER), SUB)
        g_tab[S, 0, 0] = max(g_tab[S, 0, 0], 1)  # start anchor
        g_tab[S, n_sub_here - 1, 3] = max(g_tab[S, n_sub_here - 1, 3], 1)  # stop

    # program order: groups of SGROUP supertiles, runs inside, supertiles inside
    calls = []
    chunk_off = 0
    groups = [
        list(range(g, min(g + SGROUP, n_super))) for g in range(0, n_super, SGROUP)
    ]
    for grp in groups:
        for r in range(N_RUNS):
            for S in grp:
                wS = min(SUPER, NPC - S * SUPER)
                n_sub_here = _ceil_div(wS, SUB)
                meta = []
                for so in range(n_sub_here):
                    meta.extend([so] * int(g_tab[S, so, r]))
                if not meta:
                    continue
                calls.append(
                    dict(
                        r=r,
                        S=S,
                        n_chunks=len(meta),
                        chunk_off=chunk_off,
                        meta=meta,
                        first=(r == 0),
                        last=(r == 3),
                    )
                )
                chunk_off += len(meta)
    nchunk = chunk_off
    e_pad = nchunk * 128
    gmax = max(cl["n_chunks"] for cl in calls)

    # start/stop flags: start on S's first chunk (r==0 first call for S),
    # stop on S's last chunk (r==3 last call for S). Calls for S are in
    # r order, so 'first'/'last' flags above identify them.
    # greedy queue balance + ew-engine split
    qload = [0, 0, 0, 0]
    n_sc = 0
    for i, cl in enumerate(calls):
        q = min(range(4), key=lambda k: qload[k])
        cl["queue"] = q
        qload[q] += cl["n_chunks"]
        cl["ew_sc"] = (i * EWSC) // 100 != ((i + 1) * EWSC) // 100
        n_sc += int(cl["ew_sc"])

    in_maps = []
    xT = np.zeros((NFEAT, NPAD), BF16)
    xT[:, :N_NODES] = np.asarray(x, np.float32).T.astype(BF16)
    w_bf = np.ascontiguousarray(np.asarray(W, np.float32).astype(BF16))
    bias = np.asarray(b, np.float32).reshape(nhid, 1).copy()
    iota = np.tile(np.arange(SUB, dtype=np.float32).astype(BF16)[None, :], (128, 1))

    for c in range(N_CORES):
        s_c, d_c, w_c, key = per_core[c]
        idx_pad = np.zeros(e_pad, np.int16)
        dl_pad = np.zeros(e_pad, BF16)
        ew_pad = np.zeros(e_pad, np.float32)
        uniq, first = np.unique(key, return_index=True)
        uniq = uniq.tolist()
        first = first.tolist()
        total = len(s_c)

        import bisect

        def seg(S, r, so):
            k = (S * N_RUNS + r) * max_so + so
            i = bisect.bisect_left(uniq, k)
            if i >= len(uniq) or uniq[i] != k:
                return 0, 0
            a = first[i]
            bnd = first[i + 1] if i + 1 < len(uniq) else total
            return a, bnd

        for cl in calls:
            r, S = cl["r"], cl["S"]
            run0, _ = _run_rows(r)
            pos = cl["chunk_off"] * 128
            prev_so = -1
            for so in sorted(set(cl["meta"])):
                a, bnd = seg(S, r, so)
                n = bnd - a
                capn = cl["meta"].count(so) * 128
                assert n <= capn, (c, S, r, so, n, capn)
                p0 = cl["chunk_off"] * 128 + cl["meta"].index(so) * 128
                idx_pad[p0 : p0 + n] = (s_c[a:bnd] - run0).astype(np.int16)
                dl_pad[p0 : p0 + n] = (d_c[a:bnd] & (SUB - 1)).astype(BF16)
                ew_pad[p0 : p0 + n] = w_c[a:bnd].astype(np.float32)

        in_maps.append(
            dict(
                xT=np.ascontiguousarray(xT[:, c * SHARD : (c + 1) * SHARD])
                if USE_AG
                else xT,
                wmat=w_bf,
                bias=bias,
                iota=iota,
                idx=np.ascontiguousarray(np.tile(idx_pad.reshape(-1, 16).T, (8, 1))),
                dstloc=np.ascontiguousarray(dl_pad.reshape(-1, 128).T),
                ew=np.ascontiguousarray(ew_pad.reshape(-1, 128).T),
            )
        )

    cfg = dict(
        nfeat=NFEAT,
        nhid=NHID,
        n_cores=N_CORES,
        npc=NPC,
        n_super=n_super,
        nchunk=nchunk,
        e_pad=e_pad,
        gmax=gmax,
        calls=calls,
        groups=groups,
    )
    return cfg, in_maps


def build_bass(cfg):
    F, H = cfg["nfeat"], cfg["nhid"]
    KC = F // 128
    n_super = cfg["n_super"]
    XBLK = 512

    nc = bacc.Bacc(
        "TRN2",
        target_bir_lowering=False,
        debug=False,
        enable_asserts=True,
        num_devices=cfg["n_cores"],
        num_swdge_queues=4,
    )
    f32, bf16, i16 = mybir.dt.float32, mybir.dt.bfloat16, mybir.dt.int16
    xT = nc.dram_tensor(
        "xT", [F, SHARD if USE_AG else NPAD], bf16, kind="ExternalInput"
    )
    wmat = nc.dram_tensor("wmat", [F, H], bf16, kind="ExternalInput")
    bias = nc.dram_tensor("bias", [H, 1], f32, kind="ExternalInput")
    iota = nc.dram_tensor("iota", [128, SUB], bf16, kind="ExternalInput")
    idx = nc.dram_tensor("idx", [128, cfg["e_pad"] // 16], i16, kind="ExternalInput")
    dstloc = nc.dram_tensor("dstloc", [128, cfg["nchunk"]], bf16, kind="ExternalInput")
    ew = nc.dram_tensor("ew", [128, cfg["nchunk"]], f32, kind="ExternalInput")
    outT = nc.dram_tensor("outT", [H, cfg["npc"]], f32, kind="ExternalOutput")

    AF = mybir.ActivationFunctionType
    ALU = mybir.AluOpType
    rg = [list(range(cfg["n_cores"]))]

    with tile.TileContext(nc) as tc:
        with (
            tc.tile_pool(name="dram", bufs=1, space="DRAM") as dpool,
            tc.tile_pool(name="const", bufs=1) as cpool,
            tc.tile_pool(name="xt", bufs=3) as xpool,
            tc.tile_pool(name="sup", bufs=3) as spool,
            tc.tile_pool(name="gath", bufs=8) as gpool,
            tc.tile_pool(name="gtw", bufs=4) as wpool,
            tc.tile_pool(name="ind", bufs=6) as ipool,
            tc.tile_pool(name="meta", bufs=8) as mpool,
            tc.tile_pool(name="outb", bufs=2) as opool,
            tc.tile_pool(name="ps", bufs=8, space="PSUM") as ppool,
        ):
            nc.gpsimd.load_library(_mlp_lib)
            w_sb = cpool.tile([128, KC, H], bf16)
            nc.sync.dma_start(
                out=w_sb[:], in_=wmat.ap().rearrange("(c k) h -> k c h", k=128)
            )
            bias_sb = cpool.tile([H, 1], f32)
            nc.sync.dma_start(out=bias_sb[:], in_=bias.ap())
            iota_sb = cpool.tile([128, SUB], bf16)
            nc.sync.dma_start(out=iota_sb[:], in_=iota.ap())

            if USE_AG:
                ag_in = dpool.tile([SHARD, H], bf16, name="agin", tag="agin")
            support = dpool.tile([NPAD, H], bf16, name="supp", tag="supp")

            # ---- phase 1: support = x @ W (sharded when USE_AG) ----
            n_blocks = (SHARD if USE_AG else NPAD) // XBLK
            for blk in range(n_blocks):
                xts = []
                for kc in range(KC):
                    xt = xpool.tile([128, XBLK], bf16, tag=f"xt{kc}")
                    nc.sync.dma_start(
                        out=xt[:],
                        in_=xT.ap()[
                            kc * 128 : (kc + 1) * 128, blk * XBLK : (blk + 1) * XBLK
                        ],
                    )
                    xts.append(xt)
                st = spool.tile([128, XBLK], bf16)
                ps1 = ppool.tile([128, XBLK], f32, tag="agg")
                n_col = XBLK // 128
                for i in range(n_col):
                    col = i * 128
                    for kc in range(KC):
                        nc.tensor.matmul(
                            ps1[:, col : col + 128],
                            xts[kc][:, col : col + 128],
                            w_sb[:, kc, :],
                            start=(i == 0 and kc == 0),
                            stop=(i == n_col - 1 and kc == KC - 1),
                        )
                nc.scalar.activation(out=st[:], in_=ps1[:], func=AF.Copy)
                wr_dst = ag_in if USE_AG else support
                nc.sync.dma_start(
                    out=wr_dst[blk * XBLK : (blk + 1) * XBLK, :].rearrange(
                        "(i p) h -> p i h", p=128
                    ),
                    in_=st[:].rearrange("p (i h) -> p i h", h=H),
                )
            if USE_AG:
                nc.gpsimd.collective_compute(
                    "AllGather",
                    ALU.bypass,
                    replica_groups=rg,
                    ins=[ag_in.opt()],
                    outs=[support.opt()],
                )

            # ---- phase 2: per (supertile, run) gather + one-hot matmuls ----
            # hoist the num_idxs registers (few distinct values) so the
            # per-call MOVE doesn't WAR-serialize the gather stream
            lregs = {}
            for cl in cfg["calls"]:
                L = cl["n_chunks"] * 128
                if L not in lregs:
                    lregs[L] = nc.gpsimd.to_reg(L)
            pss = {}
            for cl in cfg["calls"]:
                r, S = cl["r"], cl["S"]
                run0, run1 = _run_rows(r)
                Gc = cl["n_chunks"]
                L = Gc * 128
                c0 = cl["chunk_off"] * 8
                idxt = mpool.tile([128, L // 16], i16, tag="idx")
                nc.sync.dma_start(out=idxt[:], in_=idx.ap()[:, c0 : c0 + L // 16])
                dlt = mpool.tile([128, Gc], bf16, tag="dl")
                nc.sync.dma_start(
                    out=dlt[:],
                    in_=dstloc.ap()[:, cl["chunk_off"] : cl["chunk_off"] + Gc],
                )
                ewt = mpool.tile([128, Gc], f32, tag="ew")
                nc.sync.dma_start(
                    out=ewt[:], in_=ew.ap()[:, cl["chunk_off"] : cl["chunk_off"] + Gc]
                )
                gt = gpool.tile([128, cfg["gmax"], H], bf16, tag="gt")
                nc.gpsimd.dma_gather(
                    gt[:, :Gc, :],
                    support[run0:run1, :],
                    idxt[:],
                    L,
                    lregs[L],
                    H,
                    single_packet=False,
                    queue_num=cl["queue"],
                )
                if cl["first"]:
                    pss[S] = ppool.tile([128, SUPER], f32, tag="agg", name=f"psS{S}")
                ps = pss[S]
                ind = ipool.tile([128, Gc, SUB], bf16, tag="ind")
                nc.vector.tensor_tensor(
                    out=ind[:],
                    in0=iota_sb[:][:, None, :].to_broadcast([128, Gc, SUB]),
                    in1=dlt[:][:, :, None].to_broadcast([128, Gc, SUB]),
                    op=ALU.is_equal,
                )
                if cl["ew_sc"]:
                    # ScalarE applies the per-edge weight to the indicator
                    # (keeps gt consumed only by fast matmuls, so the
                    # gather pipeline isn't gated on ScalarE)
                    indw = wpool.tile([128, cfg["gmax"], SUB], bf16, tag="indw")
                    for j in range(Gc):
                        nc.scalar.activation(
                            out=indw[:, j, :],
                            in_=ind[:, j, :],
                            func=AF.Copy,
                            scale=ewt[:, j : j + 1],
                        )
                    mm_ind = indw
                else:
                    nc.vector.tensor_tensor(
                        out=ind[:],
                        in0=ind[:],
                        in1=ewt[:][:, :, None].to_broadcast([128, Gc, SUB]),
                        op=ALU.mult,
                    )
                    mm_ind = ind
                mm_in = gt
                first_of_S = cl["first"]
                last_of_S = cl["last"]
                for j, so in enumerate(cl["meta"]):
                    nc.tensor.matmul(
                        ps[:, so * SUB : (so + 1) * SUB],
                        mm_in[:, j, :],
                        mm_ind[:, j, :],
                        start=(first_of_S and j == 0),
                        stop=(last_of_S and j == Gc - 1),
                    )
                if last_of_S:
                    wS = min(SUPER, cfg["npc"] - S * SUPER)
                    ob = opool.tile([H, SUPER], f32)
                    nc.scalar.activation(
                        out=ob[:, :wS],
                        in_=ps[:, :wS],
                        func=AF.Relu,
                        bias=bias_sb[:],
                        scale=1.0,
                    )
                    nc.sync.dma_start(
                        out=outT.ap()[:, S * SUPER : S * SUPER + wS], in_=ob[:, :wS]
                    )
                    del pss[S]
    nc.compile()
    return nc


def kernel(x, edge_src, edge_dst, edge_w, W, b):
    x = np.asarray(x)
    cfg, in_maps = prepare(x, edge_src, edge_dst, edge_w, W, b)
    nc = build_bass(cfg)
    want_trace = bool(int(os.environ.get("GNN_TRACE", "0")))
    core_ids = list(range(cfg["n_cores"]))
    if want_trace:
        try:
            res = run_bass_kernel_spmd(nc, in_maps, core_ids=core_ids, trace=True)
        except Exception as e:
            print(f"traced run failed ({e}); retrying without trace")
            res = run_bass_kernel_spmd(nc, in_maps, core_ids=core_ids, trace=False)
    else:
        res = run_bass_kernel_spmd(nc, in_maps, core_ids=core_ids, trace=False)
    kernel.last_result = res
    out = np.concatenate([r["outT"].T for r in res.results], axis=0)
    return np.ascontiguousarray(out).astype(np.float32)


kernel.last_result = None
